# revision 1
# baseline (speedup 1.0000x reference)
"""Trainium2 Bass kernel for nn_ConvBlock (Chebyshev graph conv + BatchNorm + ReLU).

Sharding: data-parallel over batch (B=8 -> 1 sample per NeuronCore).
Per core: Chebyshev recursion via indirect-DMA row gathers + PE scatter-matmuls
(edge weights folded into host-built one-hot scatter blocks), K-stacked GEMM with
host-folded power-basis weights, BatchNorm stats on device (combined across cores
on host between two launches), normalize+ReLU+transpose on device.
"""
import os, sys
sys.path.insert(0, '/opt/trn_rl_repo')
import numpy as np
from contextlib import ExitStack

import concourse.bass as bass
import concourse.tile as tile
from concourse import bacc, mybir
from concourse.bass_utils import run_bass_kernel_spmd
from concourse.masks import make_identity

B, V, E = 8, 12288, 98304
FIN, FOUT, K = 256, 256, 4
EPS = 1e-5
P = 128
GSZ = 64            # dst-group node window (S_w block width)
NVT = V // P        # 96 vtiles (group pairs)
NCH = 24            # GEMM chunks of 512 nodes
CHV = NVT // NCH    # 4 vtiles per chunk

F32 = mybir.dt.float32
AF = mybir.ActivationFunctionType

_cache = {}


def _build_schedule(edge_src, edge_dst, edge_weight):
    """Group edges by 64-node dst windows, pad each group to multiples of 128."""
    g_of_e = edge_dst // GSZ
    order = np.argsort(g_of_e, kind='stable')
    NG = V // GSZ
    counts = np.bincount(g_of_e, minlength=NG)
    sub_of_g = np.maximum(1, (counts + P - 1) // P)   # subtiles per group
    ST = int(sub_of_g.sum())
    idx_np = np.zeros((ST, P), np.int32)              # src per (subtile, lane)
    sw = np.zeros((ST, P, GSZ), np.float32)           # scatter blocks
    vt_subs = [[] for _ in range(NVT)]                # subtile ids per vtile half
    t = 0
    pos = 0
    for g in range(NG):
        eg = order[pos:pos + counts[g]]
        pos += counts[g]
        for s in range(sub_of_g[g]):
            part = eg[s * P:(s + 1) * P]
            n = len(part)
            idx_np[t, :n] = edge_src[part]
            sw[t, np.arange(n), edge_dst[part] - g * GSZ] = edge_weight[part]
            vt_subs[g // 2].append((t, g % 2))
            t += 1
    assert t == ST
    return idx_np, sw, vt_subs, ST


def _fold_weights(weight):
    # out = sum_k T_k(L) x W_k ; T0=I, T1=L, T2=2L^2-1, T3=4L^3-3L
    # power basis z_j = L^j x :  out = sum_j z_j Wf_j
    W = weight
    Wf = np.stack([W[0] - W[2], W[1] - 3.0 * W[3], 2.0 * W[2], 4.0 * W[3]])
    # [(j,i), o] -> tiles [8, 128, 256]
    return Wf.reshape(K * FIN, FOUT).reshape(8, P, FOUT).copy()


def _build_launch_a(ST, vt_subs):
    nc = bacc.Bacc("TRN2", target_bir_lowering=False, debug=False, num_devices=8)
    xb = nc.dram_tensor("xb", [V, FIN], F32, kind="ExternalInput").ap()
    msg0 = nc.dram_tensor("msg0", [P, ST, FIN], F32, kind="ExternalInput").ap()
    idx = nc.dram_tensor("idx", [P, ST], mybir.dt.int32, kind="ExternalInput").ap()
    swt = nc.dram_tensor("swt", [P, ST * GSZ], F32, kind="ExternalInput").ap()
    wf = nc.dram_tensor("wf", [8, P, FOUT], F32, kind="ExternalInput").ap()
    rawT = nc.dram_tensor("rawT", [2, P, V], F32, kind="ExternalOutput").ap()
    stats = nc.dram_tensor("stats", [P, 4], F32, kind="ExternalOutput").ap()
    zd = [xb] + [nc.dram_tensor(f"z{j}", [V, FIN], F32).ap() for j in (1, 2, 3)]

    with tile.TileContext(nc) as tc, ExitStack() as ctx:
        cpool = ctx.enter_context(tc.tile_pool(name="const", bufs=1))
        idx_t = cpool.tile([P, ST], mybir.dt.int32, tag="idx")
        nc.sync.dma_start(idx_t[:], idx[:, :])
        ident = cpool.tile([P, P], F32, tag="id")
        make_identity(nc, ident[:])
        wf_t = cpool.tile([P, 8 * FOUT], F32, tag="wf")
        nc.sync.dma_start(wf_t[:].rearrange("p (k o) -> p k o", k=8), wf.transpose([1, 0, 2]))

        # ---- Chebyshev (power-basis) recursion: z_j = L z_{j-1} ----
        with ExitStack() as rctx:
            swp = rctx.enter_context(tc.tile_pool(name="swp", bufs=3))
            msgp = rctx.enter_context(tc.tile_pool(name="msgp", bufs=3))
            outp = rctx.enter_context(tc.tile_pool(name="outp", bufs=3))
            psp = rctx.enter_context(tc.tile_pool(name="psp", bufs=4, space="PSUM"))
            for j in (1, 2, 3):
                for vt in range(NVT):
                    subs = vt_subs[vt]
                    nst = len(subs)
                    t0 = subs[0][0]
                    sw_t = swp.tile([P, nst * GSZ], F32, tag="sw")
                    nc.sync.dma_start(sw_t[:], swt[:, t0 * GSZ:(t0 + nst) * GSZ])
                    msg_t = msgp.tile([P, nst * FIN], F32, tag="msg")
                    if j == 1:
                        nc.sync.dma_start(
                            msg_t[:].rearrange("p (t f) -> p t f", t=nst),
                            msg0[:, t0:t0 + nst, :])
                    else:
                        for s in range(nst):
                            nc.gpsimd.indirect_dma_start(
                                out=msg_t[:, s * FIN:(s + 1) * FIN], out_offset=None,
                                in_=zd[j - 1][:, :],
                                in_offset=bass.IndirectOffsetOnAxis(
                                    ap=idx_t[:, t0 + s:t0 + s + 1], axis=0))
                    ps = psp.tile([P, FIN], F32, tag="acc")
                    half_count = [sum(1 for _, h in subs if h == hh) for hh in (0, 1)]
                    seen = [0, 0]
                    for s, (t, h) in enumerate(subs):
                        nc.tensor.matmul(
                            ps[h * GSZ:(h + 1) * GSZ, :],
                            sw_t[:, s * GSZ:(s + 1) * GSZ],
                            msg_t[:, s * FIN:(s + 1) * FIN],
                            start=(seen[h] == 0), stop=(seen[h] == half_count[h] - 1))
                        seen[h] += 1
                    o_t = outp.tile([P, FIN], F32, tag="zo")
                    nc.scalar.activation(o_t[:], ps[:], AF.Copy)
                    nc.sync.dma_start(
                        zd[j].rearrange("(vt p) f -> vt p f", p=P)[vt], o_t[:])

        # ---- GEMM + BN stats ----
        with ExitStack() as gctx:
            zin = gctx.enter_context(tc.tile_pool(name="zin", bufs=2))
            ztp = gctx.enter_context(tc.tile_pool(name="ztp", bufs=2))
            big = gctx.enter_context(tc.tile_pool(name="big", bufs=1))
            psT = gctx.enter_context(tc.tile_pool(name="psT", bufs=4, space="PSUM"))
            psG = gctx.enter_context(tc.tile_pool(name="psG", bufs=2, space="PSUM"))
            rawT_sb = big.tile([P, 2 * V], F32, tag="rawT")
            stats_sb = big.tile([P, 2 * NCH * 6], F32, tag="stats")
            for c in range(NCH):
                zin_t = zin.tile([P, K * CHV * FIN], F32, tag="zin")
                for j in range(K):
                    nc.sync.dma_start(
                        zin_t[:, j * CHV * FIN:(j + 1) * CHV * FIN]
                        .rearrange("p (vt f) -> p vt f", vt=CHV),
                        zd[j].rearrange("(c vt p) f -> c p vt f", vt=CHV, p=P)[c])
                zT = ztp.tile([P, 8 * 512], F32, tag="zT")
                for j in range(K):
                    for vt in range(CHV):
                        for fh in range(2):
                            pt = psT.tile([P, P], F32, tag="pt")
                            nc.tensor.transpose(
                                pt[:],
                                zin_t[:, (j * CHV + vt) * FIN + fh * P:
                                      (j * CHV + vt) * FIN + fh * P + P],
                                ident[:])
                            kt = j * 2 + fh
                            eng = nc.vector if (vt + fh) % 2 == 0 else nc.scalar
                            if eng is nc.vector:
                                nc.vector.tensor_copy(zT[:, kt * 512 + vt * P: kt * 512 + vt * P + P], pt[:])
                            else:
                                nc.scalar.activation(zT[:, kt * 512 + vt * P: kt * 512 + vt * P + P], pt[:], AF.Copy)
                for oh in range(2):
                    pg = psG.tile([P, 512], F32, tag="pg")
                    for kt in range(8):
                        nc.tensor.matmul(
                            pg[:], wf_t[:, kt * FOUT + oh * P: kt * FOUT + oh * P + P],
                            zT[:, kt * 512:(kt + 1) * 512],
                            start=(kt == 0), stop=(kt == 7))
                    nc.vector.bn_stats(stats_sb[:, (oh * NCH + c) * 6:(oh * NCH + c) * 6 + 6], pg[:])
                    nc.scalar.activation(rawT_sb[:, oh * V + c * 512: oh * V + (c + 1) * 512], pg[:], AF.Copy)
            for oh in range(2):
                nc.sync.dma_start(rawT[oh], rawT_sb[:, oh * V:(oh + 1) * V])
            aggr = big.tile([P, 4], F32, tag="aggr")
            for oh in range(2):
                nc.vector.bn_aggr(aggr[:, oh * 2:oh * 2 + 2],
                                  stats_sb[:, oh * NCH * 6:(oh + 1) * NCH * 6])
            # stats out: [mean_h0, ex2_h0, mean_h1, ex2_h1]
            so = big.tile([P, 4], F32, tag="so")
            for oh in range(2):
                m = aggr[:, oh * 2:oh * 2 + 1]
                v_ = aggr[:, oh * 2 + 1:oh * 2 + 2]
                nc.vector.tensor_copy(so[:, oh * 2:oh * 2 + 1], m)
                nc.vector.tensor_tensor(out=so[:, oh * 2 + 1:oh * 2 + 2], in0=m, in1=m,
                                        op=mybir.AluOpType.mult)
                nc.vector.tensor_tensor(out=so[:, oh * 2 + 1:oh * 2 + 2],
                                        in0=so[:, oh * 2 + 1:oh * 2 + 2], in1=v_,
                                        op=mybir.AluOpType.add)
            nc.sync.dma_start(stats[:, :], so[:])
    nc.compile()
    return nc


def _build_launch_b():
    nc = bacc.Bacc("TRN2", target_bir_lowering=False, debug=False, num_devices=8)
    rawT = nc.dram_tensor("rawT", [2, P, V], F32, kind="ExternalInput").ap()
    sc = nc.dram_tensor("sc", [P, 2], F32, kind="ExternalInput").ap()
    sh = nc.dram_tensor("sh", [P, 2], F32, kind="ExternalInput").ap()
    out = nc.dram_tensor("out", [V, FOUT], F32, kind="ExternalOutput").ap()
    CH2 = 8           # vtiles per chunk
    NC2 = NVT // CH2  # 12 chunks
    with tile.TileContext(nc) as tc, ExitStack() as ctx:
        cpool = ctx.enter_context(tc.tile_pool(name="const", bufs=1))
        ident = cpool.tile([P, P], F32, tag="id")
        make_identity(nc, ident[:])
        sc_t = cpool.tile([P, 2], F32, tag="sc")
        sh_t = cpool.tile([P, 2], F32, tag="sh")
        nc.sync.dma_start(sc_t[:], sc[:, :])
        nc.sync.dma_start(sh_t[:], sh[:, :])
        pool = ctx.enter_context(tc.tile_pool(name="sb", bufs=2))
        psp = ctx.enter_context(tc.tile_pool(name="ps", bufs=4, space="PSUM"))
        for c in range(NC2):
            nt = pool.tile([P, 2 * CH2 * P], F32, tag="nt")
            for oh in range(2):
                nc.sync.dma_start(nt[:, oh * CH2 * P:(oh + 1) * CH2 * P],
                                  rawT[oh][:, c * CH2 * P:(c + 1) * CH2 * P])
            for oh in range(2):
                nc.scalar.activation(
                    nt[:, oh * CH2 * P:(oh + 1) * CH2 * P],
                    nt[:, oh * CH2 * P:(oh + 1) * CH2 * P],
                    AF.Relu, bias=sh_t[:, oh:oh + 1], scale=sc_t[:, oh:oh + 1])
            ot = pool.tile([P, CH2 * FOUT], F32, tag="ot")
            for vt in range(CH2):
                pt = psp.tile([P, FOUT], F32, tag="pt")
                for oh in range(2):
                    nc.tensor.transpose(
                        pt[:, oh * P:(oh + 1) * P],
                        nt[:, oh * CH2 * P + vt * P: oh * CH2 * P + (vt + 1) * P],
                        ident[:])
                eng = vt % 2
                if eng == 0:
                    nc.vector.tensor_copy(ot[:, vt * FOUT:(vt + 1) * FOUT], pt[:])
                else:
                    nc.scalar.activation(ot[:, vt * FOUT:(vt + 1) * FOUT], pt[:], AF.Copy)
            nc.sync.dma_start(
                out.rearrange("(c vt p) f -> c p vt f", vt=CH2, p=P)[c],
                ot[:].rearrange("p (vt f) -> p vt f", vt=CH2))
    nc.compile()
    return nc


def kernel(x, edge_weight, weight, bias, gamma, beta, edge_src, edge_dst):
    x = np.asarray(x, np.float32)
    edge_weight = np.asarray(edge_weight, np.float32)
    weight = np.asarray(weight, np.float32)
    gamma = np.asarray(gamma, np.float32)
    beta = np.asarray(beta, np.float32)
    edge_src = np.asarray(edge_src, np.int32)
    edge_dst = np.asarray(edge_dst, np.int32)

    idx_np, sw, vt_subs, ST = _build_schedule(edge_src, edge_dst, edge_weight)
    key = ("A", ST, tuple(len(s) for s in vt_subs))
    if key not in _cache:
        _cache[key] = _build_launch_a(ST, vt_subs)
    ncA = _cache[key]
    if "B" not in _cache:
        _cache["B"] = _build_launch_b()
    ncB = _cache["B"]

    wf = _fold_weights(weight)
    swt = np.ascontiguousarray(sw.transpose(1, 0, 2)).reshape(P, ST * GSZ)
    idx_t = np.ascontiguousarray(idx_np.T)             # [P, ST]
    in_maps = []
    for b in range(B):
        msg0 = x[b][idx_np.reshape(-1)].reshape(ST, P, FIN).transpose(1, 0, 2)
        in_maps.append({
            "xb": np.ascontiguousarray(x[b]),
            "msg0": np.ascontiguousarray(msg0),
            "idx": idx_t, "swt": swt, "wf": wf,
        })
    resA = run_bass_kernel_spmd(ncA, in_maps, core_ids=list(range(B)))

    # host: combine BN stats across cores (equal counts -> simple average)
    st = np.stack([resA.results[b]["stats"] for b in range(B)])   # [B, 128, 4]
    mean = st[:, :, [0, 2]].mean(0)                               # [128, 2]
    ex2 = st[:, :, [1, 3]].mean(0)
    var = ex2 - mean * mean
    g2 = gamma.reshape(2, P).T                                    # [128, 2]
    b2 = beta.reshape(2, P).T
    scale = (g2 / np.sqrt(var + EPS)).astype(np.float32)
    shift = (b2 - mean * scale).astype(np.float32)

    in_maps_b = [{"rawT": resA.results[b]["rawT"], "sc": scale, "sh": shift}
                 for b in range(B)]
    resB = run_bass_kernel_spmd(ncB, in_maps_b, core_ids=list(range(B)))
    global _last_inmaps
    _last_inmaps = {key: in_maps, "B": in_maps_b}
    out = np.stack([resB.results[b]["out"] for b in range(B)])
    # bias cancels inside training-mode BN (shifts the mean only); gamma/beta applied above
    return out.astype(np.float32)



# revision 21
# speedup vs baseline: 4.6156x; 4.6156x over previous
"""Trainium2 Bass kernel for nn_ConvBlock (Chebyshev graph conv + BatchNorm + ReLU).

Sharding: data-parallel over batch (B=8 -> 1 sample per NeuronCore).
Single launch per core:
  1. Chebyshev power-basis recursion z_j = L z_{j-1} (j=1..3) in bf16:
     j=1 messages host-pregathered (contiguous stream); j=2,3 via batched
     dma_gather row-gathers (1024 rows per SWDGE instruction = HW max),
     PE scatter-matmuls with host-built one-hot scatter blocks.
  2. K-stacked GEMM with host-folded power-basis weights (bf16, PE
     transposes to feature-major) + BN partial stats (fp32 PSUM).
  3. On-device AllReduce of BN stats across the 8 cores, scale/shift
     computation, fused scale+shift+ReLU, feature-major fp32 output
     (host transposes back to node-major).
"""
import sys
sys.path.insert(0, '/opt/trn_rl_repo')
import numpy as np
from contextlib import ExitStack

import ml_dtypes

import concourse.bass as bass
import concourse.tile as tile
from concourse import bacc, mybir
from concourse.bass_utils import run_bass_kernel_spmd
from concourse.masks import make_identity

B, V, E = 8, 12288, 98304
FIN, FOUT, K = 256, 256, 4
EPS = 1e-5
P = 128
GSZ = 64            # dst-group node window (S_w block width)
NVT = V // P        # 96 vtiles (group pairs)
GW = 8              # subtiles per gather window (1024 descs = HW max)
NBW = 8             # vtiles per z-write batch
NCH = 24            # GEMM chunks of 512 nodes
CHV = NVT // NCH    # 4 vtiles per chunk

F32 = mybir.dt.float32
BF16 = mybir.dt.bfloat16
I16 = mybir.dt.int16
AF = mybir.ActivationFunctionType
ALU = mybir.AluOpType
BF16_NP = ml_dtypes.bfloat16

_cache = {}


def _build_schedule(edge_src, edge_dst, edge_weight):
    """Mixed-width packing per 128-node vtile: full 64-wide subtiles per
    half-group + 128-wide cleanup subtiles holding both halves' remainders
    (emitted first so their start=True matmul initializes the whole PSUM)."""
    vt_of_e = edge_dst // P
    order = np.argsort(vt_of_e, kind='stable')
    counts = np.bincount(vt_of_e, minlength=NVT)
    idx_rows = []          # [P] int16 src per subtile
    sw_blocks = []         # [P, w] fp32 scatter block per subtile
    meta = []              # (vt, width, half) per subtile; half=-1 for cleanup
    vt_subs = [[] for _ in range(NVT)]
    pos = 0
    for vt in range(NVT):
        ev = order[pos:pos + counts[vt]]
        pos += counts[vt]
        rel = edge_dst[ev] - vt * P                   # 0..127
        a, b_ = ev[rel < GSZ], ev[rel >= GSZ]
        fla, flb = len(a) // P, len(b_) // P
        clean = np.concatenate([a[fla * P:], b_[flb * P:]])
        ncl = max((len(clean) + P - 1) // P,
                  1 if (fla == 0 or flb == 0) else 0)
        for s in range(ncl):                          # cleanups first
            part = clean[s * P:(s + 1) * P]
            blk = np.zeros((P, P), np.float32)
            row = np.zeros(P, np.int16)
            n = len(part)
            row[:n] = edge_src[part]
            blk[np.arange(n), edge_dst[part] - vt * P] = edge_weight[part]
            meta.append((vt, P, -1))
            vt_subs[vt].append(len(idx_rows))
            idx_rows.append(row)
            sw_blocks.append(blk)
        for h, full in ((0, a[:fla * P]), (1, b_[:flb * P])):
            for s in range(len(full) // P):
                part = full[s * P:(s + 1) * P]
                blk = np.zeros((P, GSZ), np.float32)
                row = np.zeros(P, np.int16)
                row[:] = edge_src[part]
                blk[np.arange(P),
                    edge_dst[part] - vt * P - h * GSZ] = edge_weight[part]
                meta.append((vt, GSZ, h))
                vt_subs[vt].append(len(idx_rows))
                idx_rows.append(row)
                sw_blocks.append(blk)
    ST = len(idx_rows)
    idx_np = np.stack(idx_rows)                       # [ST, P]
    return idx_np, sw_blocks, meta, vt_subs, ST


def _fold_weights(weight):
    # out = sum_k T_k(L) x W_k ; T0=I, T1=L, T2=2L^2-1, T3=4L^3-3L
    # power basis z_j = L^j x :  out = sum_j z_j Wf_j
    W = weight
    Wf = np.stack([W[0] - W[2], W[1] - 3.0 * W[3], 2.0 * W[2], 4.0 * W[3]])
    # [(j,fh), 128, FOUT]
    return Wf.reshape(K * FIN, FOUT).reshape(8, P, FOUT).astype(BF16_NP)


def _build_launch(ST, vt_subs):
    t2vh = [None] * ST
    for vt, subs in enumerate(vt_subs):
        for (t, h) in subs:
            t2vh[t] = (vt, h)
    NGW = (ST + GW - 1) // GW          # gather windows per apply

    nc = bacc.Bacc("TRN2", target_bir_lowering=False, debug=False, num_devices=8)
    xb = nc.dram_tensor("xb", [V, FIN], BF16, kind="ExternalInput").ap()
    xbT = nc.dram_tensor("xbT", [2, P, V], BF16, kind="ExternalInput").ap()
    msg0 = nc.dram_tensor("msg0", [P, ST * FIN], BF16, kind="ExternalInput").ap()
    idx = nc.dram_tensor("idx", [P, ST * 8], I16, kind="ExternalInput").ap()
    swt = nc.dram_tensor("swt", [P, ST * GSZ], BF16, kind="ExternalInput").ap()
    wf = nc.dram_tensor("wf", [8, P, FOUT], BF16, kind="ExternalInput").ap()
    gb = nc.dram_tensor("gb", [P, 4], F32, kind="ExternalInput").ap()
    outT = nc.dram_tensor("outT", [2, P, V], F32, kind="ExternalOutput").ap()
    zd = [xb] + [nc.dram_tensor(f"z{j}", [V, FIN], BF16).ap() for j in (1, 2, 3)]

    with tile.TileContext(nc) as tc, ExitStack() as ctx:
        cpool = ctx.enter_context(tc.tile_pool(name="const", bufs=1))
        dram = ctx.enter_context(tc.tile_pool(name="dr", bufs=1, space="DRAM"))
        ident = cpool.tile([P, P], BF16, tag="id")
        make_identity(nc, ident[:])
        wf_t = cpool.tile([P, 8 * FOUT], BF16, tag="wf")
        nc.sync.dma_start(wf_t[:].rearrange("p (k o) -> p k o", k=8),
                          wf.transpose([1, 0, 2]))
        gb_t = cpool.tile([P, 4], F32, tag="gb")
        nc.sync.dma_start(gb_t[:], gb[:, :])

        idx_t = cpool.tile([P, ST * 8], I16, tag="idx")
        nc.sync.dma_start(idx_t[:], idx[:, :])
        colw_t = cpool.tile([P, 2 * ST], F32, tag="colw")
        nc.sync.dma_start(colw_t[:], colw[:, :])
        iota_t = cpool.tile([P, P], F16, tag="iota")
        nc.sync.dma_start(iota_t[:], iota[:, :])

        wnd = []
        for g in range(NGW):
            t_lo, t_hi = g * GW, min((g + 1) * GW, ST)
            wnd.append((t_lo, t_hi, int(offs[t_lo])))
        WMAX = max(int(offs[t_hi] - offs[t_lo]) for t_lo, t_hi, _ in wnd)

        swp = ctx.enter_context(tc.tile_pool(name="swp", bufs=4))
        msgp = ctx.enter_context(tc.tile_pool(name="msgp", bufs=10))
        outp = ctx.enter_context(tc.tile_pool(name="outp", bufs=3))
        psp = ctx.enter_context(tc.tile_pool(name="psp", bufs=4, space="PSUM"))
        big = ctx.enter_context(tc.tile_pool(name="big", bufs=1))
        rawT_sb = big.tile([P, 2 * V], F16, tag="rawT")
        stats_sb = big.tile([P, 2 * NCH * 6], F32, tag="stats")
        zin = ctx.enter_context(tc.tile_pool(name="zin", bufs=2))
        ztp = ctx.enter_context(tc.tile_pool(name="ztp", bufs=2))
        psT = ctx.enter_context(tc.tile_pool(name="psT", bufs=2, space="PSUM"))
        psG = ctx.enter_context(tc.tile_pool(name="psG", bufs=2, space="PSUM"))

        def do_chunk(c, zo):
            """GEMM chunk c (512 nodes); z3 read from zo SBUF, z1/z2 from DRAM."""
            zT = ztp.tile([P, 8 * 512], F16, tag="zT", name=f"zT{c}")
            for fh in range(2):
                nc.sync.dma_start(zT[:, fh * 512:(fh + 1) * 512],
                                  xbT[fh][:, c * 512:(c + 1) * 512])
            zin_t = zin.tile([P, 2 * CHV * FIN], F16, tag="zin", name=f"zin{c}")
            for j in (1, 2):
                nc.sync.dma_start(
                    zin_t[:, (j - 1) * CHV * FIN:j * CHV * FIN]
                    .rearrange("p (vt f) -> p vt f", vt=CHV),
                    zd[j].rearrange("(c vt p) f -> c p vt f", vt=CHV, p=P)[c])
            for j in (1, 2, 3):
                for fh in range(2):
                    kt = j * 2 + fh
                    pt = psT.tile([P, 512], F16, tag="pt", name=f"pt{c}_{kt}")
                    for vt in range(CHV):
                        if j == 3:
                            src = zo[:, ((c % 2) * CHV + vt) * FIN + fh * P:
                                     ((c % 2) * CHV + vt) * FIN + fh * P + P]
                        else:
                            src = zin_t[:, ((j - 1) * CHV + vt) * FIN + fh * P:
                                        ((j - 1) * CHV + vt) * FIN + fh * P + P]
                        nc.tensor.transpose(pt[:, vt * P:(vt + 1) * P], src,
                                            ident[:])
                    if (j + fh) % 2 == 0:
                        nc.vector.tensor_copy(zT[:, kt * 512:(kt + 1) * 512],
                                              pt[:])
                    else:
                        nc.scalar.activation(zT[:, kt * 512:(kt + 1) * 512],
                                             pt[:], AF.Copy)
            for oh in range(2):
                pg = psG.tile([P, 512], F32, tag="pg", name=f"pg{c}_{oh}")
                for kt in range(8):
                    nc.tensor.matmul(
                        pg[:],
                        wf_t[:, kt * FOUT + oh * P: kt * FOUT + oh * P + P],
                        zT[:, kt * 512:(kt + 1) * 512],
                        start=(kt == 0), stop=(kt == 7))
                nc.vector.bn_stats(
                    stats_sb[:, (oh * NCH + c) * 6:(oh * NCH + c) * 6 + 6],
                    pg[:])
                nc.scalar.activation(
                    rawT_sb[:, oh * V + c * 512: oh * V + (c + 1) * 512],
                    pg[:], AF.Copy)

        # ---- Chebyshev recursion; GEMM chunks interleave into j=3 ----
        for j in (1, 2, 3):
            seen = [0] * NVT
            ps_of = [None] * NVT
            zo = None
            ncopy = 0
            for g in range(NGW):
                t_lo, t_hi, c_lo = wnd[g]
                nst = t_hi - t_lo
                msg_t = msgp.tile([P, GW * FIN], F16, tag="msg")
                if j == 1:
                    nc.sync.dma_start(msg_t[:, :nst * FIN],
                                      msg0[:, t_lo * FIN:t_hi * FIN])
                else:
                    nc.gpsimd.dma_gather(
                        out_ap=msg_t[:, :nst * FIN].rearrange(
                            "p (s f) -> p s f", s=nst),
                        in_ap=zd[j - 1][:, :],
                        idxs_ap=idx_t[:, t_lo * 8:t_hi * 8],
                        num_idxs=nst * P,
                        num_idxs_reg=nst * P,
                        elem_size=FIN)
                sw_win = swp.tile([P, WMAX], F16, tag="sww", name=f"sw{j}_{g}")
                for t in range(t_lo, t_hi):
                    w = meta[t][1]
                    o = int(offs[t]) - c_lo
                    nc.vector.tensor_scalar(
                        out=sw_win[:, o:o + w],
                        in0=iota_t[:, :w],
                        scalar1=colw_t[:, t:t + 1],
                        scalar2=colw_t[:, ST + t:ST + t + 1],
                        op0=ALU.is_equal, op1=ALU.mult)
                for t in range(t_lo, t_hi):
                    vt, w, h = meta[t]
                    if ps_of[vt] is None:
                        ps_of[vt] = psp.tile([P, FIN], F32, tag="acc",
                                             name=f"acc{j}_{vt}")
                    out_ap = (ps_of[vt][:, :] if h == -1
                              else ps_of[vt][h * GSZ:(h + 1) * GSZ, :])
                    o = int(offs[t]) - c_lo
                    nc.tensor.matmul(
                        out_ap,
                        sw_win[:, o:o + w],
                        msg_t[:, (t - t_lo) * FIN:(t - t_lo + 1) * FIN],
                        start=(t in start_t),
                        stop=(t in stop_t))
                    seen[vt] += 1
                    if seen[vt] == len(vt_subs[vt]):
                        vi = vt % NBW
                        if vi == 0:
                            zo = outp.tile([P, NBW * FIN], F16, tag="zo",
                                           name=f"zo{j}_{vt}")
                        dst = zo[:, vi * FIN:(vi + 1) * FIN]
                        if j != 2 or ncopy % 2 == 1:
                            nc.scalar.activation(dst, ps_of[vt][:], AF.Copy)
                        else:
                            nc.vector.tensor_copy(dst, ps_of[vt][:])
                        ncopy += 1
                        ps_of[vt] = None
                        if vt % NBW == NBW - 1:
                            if j != 3:
                                nc.sync.dma_start(
                                    zd[j].rearrange("(nb vt p) f -> nb p vt f",
                                                    vt=NBW, p=P)[vt // NBW],
                                    zo[:].rearrange("p (vt f) -> p vt f",
                                                    vt=NBW))
                            else:
                                do_chunk(2 * (vt // NBW), zo)
                                do_chunk(2 * (vt // NBW) + 1, zo)

        with ExitStack() as gctx:
            # ---- BN stats: local aggregate -> AllReduce -> scale/shift ----

            aggr = big.tile([P, 4], F32, tag="aggr")   # [m0, v0, m1, v1]
            for oh in range(2):
                nc.vector.bn_aggr(aggr[:, oh * 2:oh * 2 + 2],
                                  stats_sb[:, oh * NCH * 6:(oh + 1) * NCH * 6])
            sl = big.tile([P, 4], F32, tag="sl")       # [m0, m1, e0, e1]
            for oh in range(2):
                m = aggr[:, oh * 2:oh * 2 + 1]
                v_ = aggr[:, oh * 2 + 1:oh * 2 + 2]
                nc.vector.tensor_copy(sl[:, oh:oh + 1], m)
                nc.vector.tensor_tensor(out=sl[:, 2 + oh:3 + oh], in0=m, in1=m,
                                        op=ALU.mult)
                nc.vector.tensor_tensor(out=sl[:, 2 + oh:3 + oh],
                                        in0=sl[:, 2 + oh:3 + oh], in1=v_,
                                        op=ALU.add)
            cc_in = dram.tile([P, 4], F32)
            cc_out = dram.tile([8, P, 4], F32)
            nc.sync.dma_start(cc_in[:], sl[:])
            nc.gpsimd.collective_compute(
                "AllGather", ALU.bypass,
                replica_groups=[list(range(8))],
                ins=[cc_in.opt()], outs=[cc_out.opt()])
            s8 = big.tile([P, 32], F32, tag="s8")
            nc.sync.dma_start(s8[:].rearrange("p (r c) -> p r c", r=8),
                              cc_out.transpose([1, 0, 2]))
            nc.vector.tensor_tensor(out=s8[:, 0:16], in0=s8[:, 0:16],
                                    in1=s8[:, 16:32], op=ALU.add)
            nc.vector.tensor_tensor(out=s8[:, 0:8], in0=s8[:, 0:8],
                                    in1=s8[:, 8:16], op=ALU.add)
            sg = big.tile([P, 4], F32, tag="sg")
            nc.vector.tensor_tensor(out=sg[:], in0=s8[:, 0:4],
                                    in1=s8[:, 4:8], op=ALU.add)

            sc = big.tile([P, 8], F32, tag="sc")
            # cols 0-1 mean, 2-3 ex2 (scaled by 1/8); 4-5 scale, 6-7 shift
            nc.vector.tensor_scalar_mul(sc[:, 0:4], sg[:], 1.0 / 8)
            nc.vector.tensor_tensor(out=sc[:, 4:6], in0=sc[:, 0:2],
                                    in1=sc[:, 0:2], op=ALU.mult)
            nc.vector.tensor_tensor(out=sc[:, 2:4], in0=sc[:, 2:4],
                                    in1=sc[:, 4:6], op=ALU.subtract)
            nc.vector.tensor_scalar_add(sc[:, 2:4], sc[:, 2:4], EPS)
            nc.vector.reciprocal(sc[:, 2:4], sc[:, 2:4])
            nc.scalar.activation(sc[:, 2:4], sc[:, 2:4], AF.Sqrt)
            # scale = gamma * rsqrt(var+eps)
            nc.vector.tensor_tensor(out=sc[:, 4:6], in0=gb_t[:, 0:2],
                                    in1=sc[:, 2:4], op=ALU.mult)
            # shift = beta - mean*scale
            nc.vector.tensor_tensor(out=sc[:, 6:8], in0=sc[:, 0:2],
                                    in1=sc[:, 4:6], op=ALU.mult)
            nc.vector.tensor_tensor(out=sc[:, 6:8], in0=gb_t[:, 2:4],
                                    in1=sc[:, 6:8], op=ALU.subtract)

            # ---- normalize + ReLU -> outT (feature-major fp32) ----
            with ExitStack() as nctx:
                onp = nctx.enter_context(tc.tile_pool(name="onp", bufs=4))
                SLAB = 2048
                for oh in range(2):
                    for s0 in range(0, V, SLAB):
                        ot = onp.tile([P, SLAB], F32, tag="ot")
                        nc.scalar.activation(
                            ot[:], rawT_sb[:, oh * V + s0: oh * V + s0 + SLAB],
                            AF.Relu,
                            bias=sc[:, 6 + oh:7 + oh],
                            scale=sc[:, 4 + oh:5 + oh])
                        nc.sync.dma_start(outT[oh][:, s0:s0 + SLAB], ot[:])
    nc.compile()
    return nc


def kernel(x, edge_weight, weight, bias, gamma, beta, edge_src, edge_dst):
    x = np.asarray(x, np.float32)
    edge_weight = np.asarray(edge_weight, np.float32)
    weight = np.asarray(weight, np.float32)
    gamma = np.asarray(gamma, np.float32)
    beta = np.asarray(beta, np.float32)
    edge_src = np.asarray(edge_src, np.int32)
    edge_dst = np.asarray(edge_dst, np.int32)

    idx_np, sw, vt_subs, ST = _build_schedule(edge_src, edge_dst, edge_weight)
    key = (ST, tuple(len(s) for s in vt_subs))
    if key not in _cache:
        _cache[key] = _build_launch(ST, vt_subs)
    nc = _cache[key]

    wf = _fold_weights(weight)
    swt = np.ascontiguousarray(
        sw.transpose(1, 0, 2).astype(BF16_NP)).reshape(P, ST * GSZ)
    idx_flat = idx_np.reshape(-1)                      # slot i = t*128 + p
    idx_rep = np.tile(np.ascontiguousarray(idx_flat.reshape(-1, 16).T), (8, 1))
    gb = np.concatenate([gamma.reshape(2, P).T, beta.reshape(2, P).T],
                        axis=1).astype(np.float32)     # [128, 4]
    gb = np.ascontiguousarray(gb)
    in_maps = []
    for b in range(B):
        xb = np.ascontiguousarray(x[b].astype(BF16_NP))
        xbT = np.ascontiguousarray(xb.T).reshape(2, P, V)  # [2, 128, V]
        msg0 = np.ascontiguousarray(
            xb[idx_np].transpose(1, 0, 2)).reshape(P, ST * FIN)
        in_maps.append({
            "xb": xb, "xbT": xbT, "msg0": msg0,
            "idx": idx_rep, "swt": swt, "wf": wf, "gb": gb,
        })
    res = run_bass_kernel_spmd(nc, in_maps, core_ids=list(range(B)))

    out = np.empty((B, V, FOUT), np.float32)
    for b in range(B):
        oT = res.results[b]["outT"]                    # [2, 128, V] fp32
        out[b] = oT.reshape(FOUT, V).T
    # bias cancels inside training-mode BN (shifts the mean only)
    return out


# revision 24
# speedup vs baseline: 4.6339x; 1.0040x over previous
"""Trainium2 Bass kernel for nn_ConvBlock (Chebyshev graph conv + BatchNorm + ReLU).

Sharding: data-parallel over batch (B=8 -> 1 sample per NeuronCore).
Single launch per core:
  1. Chebyshev power-basis recursion z_j = L z_{j-1} (j=1..3) in bf16:
     j=1 messages host-pregathered (contiguous stream); j=2,3 via batched
     dma_gather row-gathers (1024 rows per SWDGE instruction = HW max),
     PE scatter-matmuls with host-built one-hot scatter blocks.
  2. K-stacked GEMM with host-folded power-basis weights (bf16, PE
     transposes to feature-major) + BN partial stats (fp32 PSUM).
  3. On-device AllReduce of BN stats across the 8 cores, scale/shift
     computation, fused scale+shift+ReLU, feature-major fp32 output
     (host transposes back to node-major).
"""
import sys
sys.path.insert(0, '/opt/trn_rl_repo')
import numpy as np
from contextlib import ExitStack

import ml_dtypes

import concourse.bass as bass
import concourse.tile as tile
from concourse import bacc, mybir
from concourse.bass_utils import run_bass_kernel_spmd
from concourse.masks import make_identity

B, V, E = 8, 12288, 98304
FIN, FOUT, K = 256, 256, 4
EPS = 1e-5
P = 128
GSZ = 64            # dst-group node window (S_w block width)
NVT = V // P        # 96 vtiles (group pairs)
GW = 8              # subtiles per gather window (1024 descs = HW max)
NBW = 8             # vtiles per z-write batch
NCH = 24            # GEMM chunks of 512 nodes
CHV = NVT // NCH    # 4 vtiles per chunk

F32 = mybir.dt.float32
BF16 = mybir.dt.bfloat16
I16 = mybir.dt.int16
AF = mybir.ActivationFunctionType
ALU = mybir.AluOpType
BF16_NP = ml_dtypes.bfloat16

_cache = {}


def _build_schedule(edge_src, edge_dst, edge_weight):
    """Mixed-width packing per 128-node vtile: full 64-wide subtiles per
    half-group + 128-wide cleanup subtiles holding both halves' remainders
    (emitted first so their start=True matmul initializes the whole PSUM)."""
    vt_of_e = edge_dst // P
    order = np.argsort(vt_of_e, kind='stable')
    counts = np.bincount(vt_of_e, minlength=NVT)
    idx_rows = []          # [P] int16 src per subtile
    sw_blocks = []         # [P, w] fp32 scatter block per subtile
    meta = []              # (vt, width, half) per subtile; half=-1 for cleanup
    vt_subs = [[] for _ in range(NVT)]
    pos = 0
    for vt in range(NVT):
        ev = order[pos:pos + counts[vt]]
        pos += counts[vt]
        rel = edge_dst[ev] - vt * P                   # 0..127
        a, b_ = ev[rel < GSZ], ev[rel >= GSZ]
        fla, flb = len(a) // P, len(b_) // P
        clean = np.concatenate([a[fla * P:], b_[flb * P:]])
        ncl = max((len(clean) + P - 1) // P,
                  1 if (fla == 0 or flb == 0) else 0)
        for s in range(ncl):                          # cleanups first
            part = clean[s * P:(s + 1) * P]
            blk = np.zeros((P, P), np.float32)
            row = np.zeros(P, np.int16)
            n = len(part)
            row[:n] = edge_src[part]
            blk[np.arange(n), edge_dst[part] - vt * P] = edge_weight[part]
            meta.append((vt, P, -1))
            vt_subs[vt].append(len(idx_rows))
            idx_rows.append(row)
            sw_blocks.append(blk)
        for h, full in ((0, a[:fla * P]), (1, b_[:flb * P])):
            for s in range(len(full) // P):
                part = full[s * P:(s + 1) * P]
                blk = np.zeros((P, GSZ), np.float32)
                row = np.zeros(P, np.int16)
                row[:] = edge_src[part]
                blk[np.arange(P),
                    edge_dst[part] - vt * P - h * GSZ] = edge_weight[part]
                meta.append((vt, GSZ, h))
                vt_subs[vt].append(len(idx_rows))
                idx_rows.append(row)
                sw_blocks.append(blk)
    ST = len(idx_rows)
    idx_np = np.stack(idx_rows)                       # [ST, P]
    return idx_np, sw_blocks, meta, vt_subs, ST


def _fold_weights(weight):
    # out = sum_k T_k(L) x W_k ; T0=I, T1=L, T2=2L^2-1, T3=4L^3-3L
    # power basis z_j = L^j x :  out = sum_j z_j Wf_j
    W = weight
    Wf = np.stack([W[0] - W[2], W[1] - 3.0 * W[3], 2.0 * W[2], 4.0 * W[3]])
    # [(j,fh), 128, FOUT]
    return Wf.reshape(K * FIN, FOUT).reshape(8, P, FOUT).astype(BF16_NP)


def _build_launch(ST, vt_subs):
    t2vh = [None] * ST
    for vt, subs in enumerate(vt_subs):
        for (t, h) in subs:
            t2vh[t] = (vt, h)
    NGW = (ST + GW - 1) // GW          # gather windows per apply

    nc = bacc.Bacc("TRN2", target_bir_lowering=False, debug=False, num_devices=8)
    xb = nc.dram_tensor("xb", [V, FIN], BF16, kind="ExternalInput").ap()
    xbT = nc.dram_tensor("xbT", [2, P, V], BF16, kind="ExternalInput").ap()
    msg0 = nc.dram_tensor("msg0", [P, ST * FIN], BF16, kind="ExternalInput").ap()
    idx = nc.dram_tensor("idx", [P, ST * 8], I16, kind="ExternalInput").ap()
    swt = nc.dram_tensor("swt", [P, ST * GSZ], BF16, kind="ExternalInput").ap()
    wf = nc.dram_tensor("wf", [8, P, FOUT], BF16, kind="ExternalInput").ap()
    gb = nc.dram_tensor("gb", [P, 4], F32, kind="ExternalInput").ap()
    outT = nc.dram_tensor("outT", [2, P, V], F32, kind="ExternalOutput").ap()
    zd = [xb] + [nc.dram_tensor(f"z{j}", [V, FIN], BF16).ap() for j in (1, 2, 3)]

    with tile.TileContext(nc) as tc, ExitStack() as ctx:
        cpool = ctx.enter_context(tc.tile_pool(name="const", bufs=1))
        dram = ctx.enter_context(tc.tile_pool(name="dr", bufs=1, space="DRAM"))
        ident = cpool.tile([P, P], BF16, tag="id")
        make_identity(nc, ident[:])
        wf_t = cpool.tile([P, 8 * FOUT], BF16, tag="wf")
        nc.sync.dma_start(wf_t[:].rearrange("p (k o) -> p k o", k=8),
                          wf.transpose([1, 0, 2]))
        gb_t = cpool.tile([P, 4], F32, tag="gb")
        nc.sync.dma_start(gb_t[:], gb[:, :])

        idx_t = cpool.tile([P, ST * 8], I16, tag="idx")
        nc.sync.dma_start(idx_t[:], idx[:, :])
        colw_t = cpool.tile([P, 2 * ST], F32, tag="colw")
        nc.sync.dma_start(colw_t[:], colw[:, :])
        iota_t = cpool.tile([P, P], F16, tag="iota")
        nc.sync.dma_start(iota_t[:], iota[:, :])

        wnd = []
        for g in range(NGW):
            t_lo, t_hi = g * GW, min((g + 1) * GW, ST)
            wnd.append((t_lo, t_hi, int(offs[t_lo])))
        WMAX = max(int(offs[t_hi] - offs[t_lo]) for t_lo, t_hi, _ in wnd)

        swp = ctx.enter_context(tc.tile_pool(name="swp", bufs=4))
        msgp = ctx.enter_context(tc.tile_pool(name="msgp", bufs=10))
        outp = ctx.enter_context(tc.tile_pool(name="outp", bufs=3))
        psp = ctx.enter_context(tc.tile_pool(name="psp", bufs=4, space="PSUM"))
        big = ctx.enter_context(tc.tile_pool(name="big", bufs=1))
        rawT_sb = big.tile([P, 2 * V], F16, tag="rawT")
        stats_sb = big.tile([P, 2 * NCH * 6], F32, tag="stats")
        zin = ctx.enter_context(tc.tile_pool(name="zin", bufs=2))
        ztp = ctx.enter_context(tc.tile_pool(name="ztp", bufs=4))
        psT = ctx.enter_context(tc.tile_pool(name="psT", bufs=2, space="PSUM"))
        psG = ctx.enter_context(tc.tile_pool(name="psG", bufs=2, space="PSUM"))

        def prep_chunk(c):
            """xbT/z1/z2 part of GEMM chunk c (independent of j=3)."""
            zT = ztp.tile([P, 8 * 512], F16, tag="zT", name=f"zT{c}")
            for fh in range(2):
                nc.sync.dma_start(zT[:, fh * 512:(fh + 1) * 512],
                                  xbT[fh][:, c * 512:(c + 1) * 512])
            zin_t = zin.tile([P, 2 * CHV * FIN], F16, tag="zin", name=f"zin{c}")
            for j in (1, 2):
                nc.sync.dma_start(
                    zin_t[:, (j - 1) * CHV * FIN:j * CHV * FIN]
                    .rearrange("p (vt f) -> p vt f", vt=CHV),
                    zd[j].rearrange("(c vt p) f -> c p vt f", vt=CHV, p=P)[c])
            for j in (1, 2):
                for fh in range(2):
                    kt = j * 2 + fh
                    pt = psT.tile([P, 512], F16, tag="pt", name=f"pt{c}_{kt}")
                    for vt in range(CHV):
                        src = zin_t[:, ((j - 1) * CHV + vt) * FIN + fh * P:
                                    ((j - 1) * CHV + vt) * FIN + fh * P + P]
                        nc.tensor.transpose(pt[:, vt * P:(vt + 1) * P], src,
                                            ident[:])
                    if (j + fh) % 2 == 0:
                        nc.vector.tensor_copy(zT[:, kt * 512:(kt + 1) * 512],
                                              pt[:])
                    else:
                        nc.scalar.activation(zT[:, kt * 512:(kt + 1) * 512],
                                             pt[:], AF.Copy)
            return zT

        def finish_chunk(c, zo, zT):
            """z3 transposes + K-stacked GEMM + BN stats for chunk c."""
            for fh in range(2):
                kt = 6 + fh
                pt = psT.tile([P, 512], F16, tag="pt", name=f"pt{c}_{kt}")
                for vt in range(CHV):
                    src = zo[:, ((c % 2) * CHV + vt) * FIN + fh * P:
                             ((c % 2) * CHV + vt) * FIN + fh * P + P]
                    nc.tensor.transpose(pt[:, vt * P:(vt + 1) * P], src,
                                        ident[:])
                if fh == 0:
                    nc.scalar.activation(zT[:, kt * 512:(kt + 1) * 512],
                                         pt[:], AF.Copy)
                else:
                    nc.vector.tensor_copy(zT[:, kt * 512:(kt + 1) * 512],
                                          pt[:])
            for oh in range(2):
                pg = psG.tile([P, 512], F32, tag="pg", name=f"pg{c}_{oh}")
                for kt in range(8):
                    nc.tensor.matmul(
                        pg[:],
                        wf_t[:, kt * FOUT + oh * P: kt * FOUT + oh * P + P],
                        zT[:, kt * 512:(kt + 1) * 512],
                        start=(kt == 0), stop=(kt == 7))
                nc.vector.bn_stats(
                    stats_sb[:, (oh * NCH + c) * 6:(oh * NCH + c) * 6 + 6],
                    pg[:])
                nc.scalar.activation(
                    rawT_sb[:, oh * V + c * 512: oh * V + (c + 1) * 512],
                    pg[:], AF.Copy)

        # ---- Chebyshev recursion; GEMM chunks interleave into j=3 ----
        prepped = {}
        for j in (1, 2, 3):
            if j == 3:
                prepped[0] = prep_chunk(0)
                prepped[1] = prep_chunk(1)
            seen = [0] * NVT
            ps_of = [None] * NVT
            zo = None
            ncopy = 0
            for g in range(NGW):
                t_lo, t_hi, c_lo = wnd[g]
                nst = t_hi - t_lo
                msg_t = msgp.tile([P, GW * FIN], F16, tag="msg")
                if j == 1:
                    nc.sync.dma_start(msg_t[:, :nst * FIN],
                                      msg0[:, t_lo * FIN:t_hi * FIN])
                else:
                    nc.gpsimd.dma_gather(
                        out_ap=msg_t[:, :nst * FIN].rearrange(
                            "p (s f) -> p s f", s=nst),
                        in_ap=zd[j - 1][:, :],
                        idxs_ap=idx_t[:, t_lo * 8:t_hi * 8],
                        num_idxs=nst * P,
                        num_idxs_reg=nst * P,
                        elem_size=FIN)
                sw_win = swp.tile([P, WMAX], F16, tag="sww", name=f"sw{j}_{g}")
                for t in range(t_lo, t_hi):
                    w = meta[t][1]
                    o = int(offs[t]) - c_lo
                    nc.vector.tensor_scalar(
                        out=sw_win[:, o:o + w],
                        in0=iota_t[:, :w],
                        scalar1=colw_t[:, t:t + 1],
                        scalar2=colw_t[:, ST + t:ST + t + 1],
                        op0=ALU.is_equal, op1=ALU.mult)
                for t in range(t_lo, t_hi):
                    vt, w, h = meta[t]
                    if ps_of[vt] is None:
                        ps_of[vt] = psp.tile([P, FIN], F32, tag="acc",
                                             name=f"acc{j}_{vt}")
                    out_ap = (ps_of[vt][:, :] if h == -1
                              else ps_of[vt][h * GSZ:(h + 1) * GSZ, :])
                    o = int(offs[t]) - c_lo
                    nc.tensor.matmul(
                        out_ap,
                        sw_win[:, o:o + w],
                        msg_t[:, (t - t_lo) * FIN:(t - t_lo + 1) * FIN],
                        start=(t in start_t),
                        stop=(t in stop_t))
                    seen[vt] += 1
                    if seen[vt] == len(vt_subs[vt]):
                        vi = vt % NBW
                        if vi == 0:
                            zo = outp.tile([P, NBW * FIN], F16, tag="zo",
                                           name=f"zo{j}_{vt}")
                        dst = zo[:, vi * FIN:(vi + 1) * FIN]
                        if j != 2 or ncopy % 2 == 1:
                            nc.scalar.activation(dst, ps_of[vt][:], AF.Copy)
                        else:
                            nc.vector.tensor_copy(dst, ps_of[vt][:])
                        ncopy += 1
                        ps_of[vt] = None
                        if vt % NBW == NBW - 1:
                            if j != 3:
                                nc.sync.dma_start(
                                    zd[j].rearrange("(nb vt p) f -> nb p vt f",
                                                    vt=NBW, p=P)[vt // NBW],
                                    zo[:].rearrange("p (vt f) -> p vt f",
                                                    vt=NBW))
                            else:
                                k = vt // NBW
                                finish_chunk(2 * k, zo, prepped.pop(2 * k))
                                finish_chunk(2 * k + 1, zo,
                                             prepped.pop(2 * k + 1))
                                if k + 1 < NVT // NBW:
                                    prepped[2 * k + 2] = prep_chunk(2 * k + 2)
                                    prepped[2 * k + 3] = prep_chunk(2 * k + 3)

        with ExitStack() as gctx:
            # ---- BN stats: local aggregate -> AllReduce -> scale/shift ----

            aggr = big.tile([P, 4], F32, tag="aggr")   # [m0, v0, m1, v1]
            for oh in range(2):
                nc.vector.bn_aggr(aggr[:, oh * 2:oh * 2 + 2],
                                  stats_sb[:, oh * NCH * 6:(oh + 1) * NCH * 6])
            sl = big.tile([P, 4], F32, tag="sl")       # [m0, m1, e0, e1]
            for oh in range(2):
                m = aggr[:, oh * 2:oh * 2 + 1]
                v_ = aggr[:, oh * 2 + 1:oh * 2 + 2]
                nc.vector.tensor_copy(sl[:, oh:oh + 1], m)
                nc.vector.tensor_tensor(out=sl[:, 2 + oh:3 + oh], in0=m, in1=m,
                                        op=ALU.mult)
                nc.vector.tensor_tensor(out=sl[:, 2 + oh:3 + oh],
                                        in0=sl[:, 2 + oh:3 + oh], in1=v_,
                                        op=ALU.add)
            cc_in = dram.tile([P, 4], F32)
            cc_out = dram.tile([8, P, 4], F32)
            nc.sync.dma_start(cc_in[:], sl[:])
            nc.gpsimd.collective_compute(
                "AllGather", ALU.bypass,
                replica_groups=[list(range(8))],
                ins=[cc_in.opt()], outs=[cc_out.opt()])
            s8 = big.tile([P, 32], F32, tag="s8")
            nc.sync.dma_start(s8[:].rearrange("p (r c) -> p r c", r=8),
                              cc_out.transpose([1, 0, 2]))
            nc.vector.tensor_tensor(out=s8[:, 0:16], in0=s8[:, 0:16],
                                    in1=s8[:, 16:32], op=ALU.add)
            nc.vector.tensor_tensor(out=s8[:, 0:8], in0=s8[:, 0:8],
                                    in1=s8[:, 8:16], op=ALU.add)
            sg = big.tile([P, 4], F32, tag="sg")
            nc.vector.tensor_tensor(out=sg[:], in0=s8[:, 0:4],
                                    in1=s8[:, 4:8], op=ALU.add)

            sc = big.tile([P, 8], F32, tag="sc")
            # cols 0-1 mean, 2-3 ex2 (scaled by 1/8); 4-5 scale, 6-7 shift
            nc.vector.tensor_scalar_mul(sc[:, 0:4], sg[:], 1.0 / 8)
            nc.vector.tensor_tensor(out=sc[:, 4:6], in0=sc[:, 0:2],
                                    in1=sc[:, 0:2], op=ALU.mult)
            nc.vector.tensor_tensor(out=sc[:, 2:4], in0=sc[:, 2:4],
                                    in1=sc[:, 4:6], op=ALU.subtract)
            nc.vector.tensor_scalar_add(sc[:, 2:4], sc[:, 2:4], EPS)
            nc.vector.reciprocal(sc[:, 2:4], sc[:, 2:4])
            nc.scalar.activation(sc[:, 2:4], sc[:, 2:4], AF.Sqrt)
            # scale = gamma * rsqrt(var+eps)
            nc.vector.tensor_tensor(out=sc[:, 4:6], in0=gb_t[:, 0:2],
                                    in1=sc[:, 2:4], op=ALU.mult)
            # shift = beta - mean*scale
            nc.vector.tensor_tensor(out=sc[:, 6:8], in0=sc[:, 0:2],
                                    in1=sc[:, 4:6], op=ALU.mult)
            nc.vector.tensor_tensor(out=sc[:, 6:8], in0=gb_t[:, 2:4],
                                    in1=sc[:, 6:8], op=ALU.subtract)

            # ---- normalize + ReLU -> outT (feature-major fp32) ----
            with ExitStack() as nctx:
                onp = nctx.enter_context(tc.tile_pool(name="onp", bufs=4))
                SLAB = 2048
                for oh in range(2):
                    for s0 in range(0, V, SLAB):
                        ot = onp.tile([P, SLAB], F32, tag="ot")
                        nc.scalar.activation(
                            ot[:], rawT_sb[:, oh * V + s0: oh * V + s0 + SLAB],
                            AF.Relu,
                            bias=sc[:, 6 + oh:7 + oh],
                            scale=sc[:, 4 + oh:5 + oh])
                        nc.sync.dma_start(outT[oh][:, s0:s0 + SLAB], ot[:])
    nc.compile()
    return nc


def kernel(x, edge_weight, weight, bias, gamma, beta, edge_src, edge_dst):
    x = np.asarray(x, np.float32)
    edge_weight = np.asarray(edge_weight, np.float32)
    weight = np.asarray(weight, np.float32)
    gamma = np.asarray(gamma, np.float32)
    beta = np.asarray(beta, np.float32)
    edge_src = np.asarray(edge_src, np.int32)
    edge_dst = np.asarray(edge_dst, np.int32)

    idx_np, sw, vt_subs, ST = _build_schedule(edge_src, edge_dst, edge_weight)
    key = (ST, tuple(len(s) for s in vt_subs))
    if key not in _cache:
        _cache[key] = _build_launch(ST, vt_subs)
    nc = _cache[key]

    wf = _fold_weights(weight)
    swt = np.ascontiguousarray(
        sw.transpose(1, 0, 2).astype(BF16_NP)).reshape(P, ST * GSZ)
    idx_flat = idx_np.reshape(-1)                      # slot i = t*128 + p
    idx_rep = np.tile(np.ascontiguousarray(idx_flat.reshape(-1, 16).T), (8, 1))
    gb = np.concatenate([gamma.reshape(2, P).T, beta.reshape(2, P).T],
                        axis=1).astype(np.float32)     # [128, 4]
    gb = np.ascontiguousarray(gb)
    in_maps = []
    for b in range(B):
        xb = np.ascontiguousarray(x[b].astype(BF16_NP))
        xbT = np.ascontiguousarray(xb.T).reshape(2, P, V)  # [2, 128, V]
        msg0 = np.ascontiguousarray(
            xb[idx_np].transpose(1, 0, 2)).reshape(P, ST * FIN)
        in_maps.append({
            "xb": xb, "xbT": xbT, "msg0": msg0,
            "idx": idx_rep, "swt": swt, "wf": wf, "gb": gb,
        })
    res = run_bass_kernel_spmd(nc, in_maps, core_ids=list(range(B)))

    out = np.empty((B, V, FOUT), np.float32)
    for b in range(B):
        oT = res.results[b]["outT"]                    # [2, 128, V] fp32
        out[b] = oT.reshape(FOUT, V).T
    # bias cancels inside training-mode BN (shifts the mean only)
    return out
